# revision 1
# baseline (speedup 1.0000x reference)
"""Grouped-query attention + output projection on 8 trn2 NeuronCores.

Sharding: KV group g (and its 4 query heads) -> core g; out_proj column-
sharded (512 cols/core) with a per-q-chunk AllGather of the attention
outputs; projections are interleaved so each one lands after its
AllGather is guaranteed complete.

Per (q-chunk c, head h), all in bf16 on the tensor engine (transposed
layout, no on-device transposes):

  mm1:   scoresT[k, q] = kT_tile.T @ qT      (2 MMs into one [128,1024] PSUM)
  exp:   ACT Exp with fused 1/sqrt(D) scale, PSUM -> SBUF bf16 mega-tile
  denom: DVE fold tree (4 halving adds over the [128, 8192] exp mega-tile)
         then ONE ones-matmul [1,512]  (replaces 16 per-k-tile matmuls)
  mm2:   outT[d, q] += v_tile.T @ expT       (accumulated over k tiles)
  recip: custom-DVE reciprocal_approx_fast ([1,512], ~5x faster than exact)
  norm:  outT * (ones x recip) broadcast via K=1 f32r matmul

The collective stream pays a ~40us entry barrier before the first AG can
start, and AGs serialize at ~20-30us each, so the first AllGather is not
done before ~105us no matter what.  Emission order
  attn0 attn1 attn2 proj0 attn3 proj1 proj2 proj3
keeps the PE dense while every proj(c) arrives after AG(c) completed.
Host pre-lays out v/wT/qT in SBUF shape so resident loads are single
contiguous DMAs (dma_start issues serialize at ~0.6us on the sync queue).
"""

import sys

import numpy as np

S = 2048
H = 32
G = 8
D = 128
HPG = H // G          # 4 heads per group/core
MODEL = H * D         # 4096
NCORES = 8
MS = MODEL // NCORES  # 512 output columns per core
JS = HPG * D          # 512 concat rows per core
QC = 512              # q-chunk (matmul free dim)
NQC = S // QC         # 4
NKT = S // 128        # 16 k tiles
NKP = NKT // 2        # 8 k-tile pairs
NJT = MODEL // 128    # 32 j tiles (proj contraction)

_CACHE = {}


def _build_bass():
    if "/opt/trn_rl_repo" not in sys.path:
        sys.path.insert(0, "/opt/trn_rl_repo")
    import concourse.bacc as bacc
    import concourse.mybir as mybir
    import concourse.tile as tile
    from concourse.dve_ops import RECIP_APPROX_FAST_CONSTS, RECIPROCAL_APPROX_FAST

    f32 = mybir.dt.float32
    f32r = mybir.dt.float32r
    bf16 = mybir.dt.bfloat16
    EXP = mybir.ActivationFunctionType.Exp
    COPY = mybir.ActivationFunctionType.Copy
    scale = float(D) ** -0.5

    nc = bacc.Bacc(None, num_devices=NCORES)
    # Host pre-transposed, SBUF-shaped layouts (single contiguous DMA each).
    qT = nc.dram_tensor("qT", [128, NQC * HPG * QC], bf16, kind="ExternalInput")
    kT = nc.dram_tensor("kT", [D, S], bf16, kind="ExternalInput")
    v = nc.dram_tensor("v", [128, NKT * D], bf16, kind="ExternalInput")
    wT = nc.dram_tensor("wT", [128, NJT * MS], bf16, kind="ExternalInput")
    ones_b = nc.dram_tensor("ones_b", [128, 128], bf16, kind="ExternalInput")
    ones_f = nc.dram_tensor("ones_f", [1, 128], f32r, kind="ExternalInput")
    out = nc.dram_tensor("out", [S, MS], f32, kind="ExternalOutput")

    lp = nc.allow_low_precision("bf16 attention + projection")
    lp.__enter__()
    with tile.TileContext(nc) as tc:
        with (
            tc.tile_pool(name="const", bufs=1) as constp,
            tc.tile_pool(name="kv", bufs=1) as kvp,
            tc.tile_pool(name="w", bufs=1) as wp,
            tc.tile_pool(name="qt", bufs=2) as qtp,
            tc.tile_pool(name="expt", bufs=2) as expp,
            tc.tile_pool(name="cc", bufs=3) as ccp,
            tc.tile_pool(name="misc", bufs=2) as miscp,
            tc.tile_pool(name="proj_in", bufs=3) as pip,
            tc.tile_pool(name="out_sb", bufs=2) as outp,
            tc.tile_pool(name="ps_s", bufs=2, space="PSUM") as ps_s,
            tc.tile_pool(name="ps_acc", bufs=2, space="PSUM") as ps_acc,
            tc.tile_pool(name="ps_db", bufs=2, space="PSUM") as ps_db,
            tc.tile_pool(name="dram", bufs=1, space="DRAM") as dramp,
        ):
            # Resident operands.  kT is split so mm1 of the first k-tiles can
            # start before the whole row lands; q chunk 0 loads before v/ones
            # (sync-queue DMA issues serialize, first mm1 needs kT+q only).
            kT_sb = kvp.tile([128, S], bf16, name="kT_sb")
            nc.sync.dma_start(kT_sb[:, 0:512], kT[:, 0:512])
            v_sb = kvp.tile([128, NKT * D], bf16, name="v_sb")
            ones_sb = constp.tile([128, 128], bf16, name="ones_sb")
            onesf_sb = constp.tile([1, 128], f32r, name="onesf_sb")
            wT_sb = wp.tile([128, NJT * MS], bf16, name="wT_sb")

            def load_residents():
                nc.sync.dma_start(kT_sb[:, 512:], kT[:, 512:])
                nc.sync.dma_start(v_sb[:], v[:])
                nc.sync.dma_start(ones_sb[:], ones_b[:])
                nc.sync.dma_start(onesf_sb[:], ones_f[:])

            cc_in = []
            cc_out = []
            for c in range(NQC):
                cc_in.append(
                    dramp.tile([JS, QC], bf16, name=f"cc_in_{c}", tag=f"cci{c}")
                )
                cc_out.append(
                    dramp.tile(
                        [MODEL, QC], bf16, name=f"cc_out_{c}", tag=f"cco{c}",
                        addr_space="Shared",
                    )
                )

            def attn_chunk(c, after_q=None, head_hook=None):
                q_sb = qtp.tile([128, HPG * QC], bf16, tag="q", name="q_sb")
                base = c * HPG * QC
                if c == 0:
                    # Head 0 first so the very first mm1 isn't gated on the
                    # full 1MB chunk transfer.
                    nc.sync.dma_start(q_sb[:, 0:QC], qT[:, base : base + QC])
                    nc.sync.dma_start(
                        q_sb[:, QC:], qT[:, base + QC : base + HPG * QC]
                    )
                else:
                    nc.sync.dma_start(q_sb[:], qT[:, base : base + HPG * QC])
                if after_q is not None:
                    after_q()
                for h in range(HPG):
                    ex = expp.tile([128, NKT * QC], bf16, tag="exp", name="ex")
                    po = ps_acc.tile([128, QC], f32, tag="acc", name="po")
                    for p in range(NKP):
                        ps = ps_s.tile([128, 2 * QC], f32, tag="scores", name="ps")
                        for u in range(2):
                            t = 2 * p + u
                            nc.tensor.matmul(
                                ps[:, u * QC : (u + 1) * QC],
                                kT_sb[:, t * 128 : (t + 1) * 128],
                                q_sb[:, h * QC : (h + 1) * QC],
                                start=True,
                                stop=True,
                            )
                        nc.scalar.activation(
                            ex[:, p * 2 * QC : (p + 1) * 2 * QC],
                            ps[:],
                            EXP,
                            scale=scale,
                        )
                        for u in range(2):
                            t = 2 * p + u
                            nc.tensor.matmul(
                                po[:],
                                v_sb[:, t * D : (t + 1) * D],
                                ex[:, t * QC : (t + 1) * QC],
                                start=(t == 0),
                                stop=(t == NKT - 1),
                            )
                    # Denominator: fold tree sums the 16 k-tiles pointwise in q.
                    w_ = NKT * QC // 2
                    while w_ >= QC:
                        nc.vector.tensor_add(
                            ex[:, 0:w_], ex[:, 0:w_], ex[:, w_ : 2 * w_]
                        )
                        w_ //= 2
                    if head_hook is not None:
                        # A proj si-group (~8.4us of matmuls) emitted here
                        # covers the DVE fold latency so the PE never stalls
                        # waiting to issue the denominator matmul.
                        head_hook(h)
                    psd = ps_db.tile([1, QC], f32, tag="db", name="psd")
                    nc.tensor.matmul(
                        psd[:],
                        ones_sb[:, 0:1],
                        ex[:, 0:QC],
                        start=True,
                        stop=True,
                    )
                    rc = miscp.tile([1, QC], f32r, tag="recip", name="rc")
                    nc.vector._custom_dve(
                        RECIPROCAL_APPROX_FAST,
                        out=rc[:],
                        in0=psd[:],
                        s0=RECIP_APPROX_FAST_CONSTS["s0"],
                        s1=RECIP_APPROX_FAST_CONSTS["s1"],
                        imm2=RECIP_APPROX_FAST_CONSTS["imm2"],
                    )
                    psb = ps_db.tile([128, QC], f32, tag="db", name="psb")
                    nc.tensor.matmul(
                        psb[:], onesf_sb[0:1, :], rc[:], start=True, stop=True
                    )
                    rb = miscp.tile([128, QC], f32, tag="rb", name="rb")
                    nc.vector.tensor_copy(rb[:], psb[:])
                    cc_sb = ccp.tile([128, QC], bf16, tag="cc", name="cc_sb")
                    nc.vector.tensor_mul(cc_sb[:], po[:], rb[:])
                    nc.sync.dma_start(
                        cc_in[c][h * 128 : (h + 1) * 128, :], cc_sb[:]
                    )
                nc.gpsimd.collective_compute(
                    "AllGather",
                    mybir.AluOpType.bypass,
                    replica_groups=[list(range(NCORES))],
                    ins=[cc_in[c].opt()],
                    outs=[cc_out[c].opt()],
                )

            def proj_load(c):
                # lt free layout: j-tile a (global) at a*512.  Loaded in
                # quarters from the gpsimd queue: emitted right after this
                # chunk's AllGather, so on the in-order gpsimd queue the
                # load starts the moment the gather lands, without tying up
                # the sync queue (a 1MB strided DMA blocks its queue ~3us).
                nq = 4
                lt = pip.tile([128, NJT * QC], bf16, tag="pin", name="lt")
                step = NJT // nq
                for qtr in range(nq):
                    nc.gpsimd.dma_start(
                        lt[:, qtr * step * QC : (qtr + 1) * step * QC].rearrange(
                            "p (a q) -> p a q", a=step, q=QC
                        ),
                        cc_out[c][qtr * step * 128 : (qtr + 1) * step * 128, :]
                        .rearrange("(a p) q -> p a q", a=step, p=128),
                    )
                return lt

            def proj_si(c, lt, si):
                pp = ps_db.tile([128, MS], f32, tag="db", name="pp")
                for a in range(NJT):
                    nc.tensor.matmul(
                        pp[:],
                        lt[:, a * QC + si * 128 : a * QC + (si + 1) * 128],
                        wT_sb[:, a * MS : (a + 1) * MS],
                        start=(a == 0),
                        stop=(a == NJT - 1),
                    )
                o_sb = outp.tile([128, MS], f32, tag="o", name="o_sb")
                nc.vector.tensor_copy(o_sb[:], pp[:])
                nc.sync.dma_start(
                    out[(c * 4 + si) * 128 : (c * 4 + si + 1) * 128, :],
                    o_sb[:],
                )

            def proj_chunk(c, lt):
                for si in range(QC // 128):
                    proj_si(c, lt, si)

            attn_chunk(0, after_q=load_residents)
            nc.sync.dma_start(wT_sb[:], wT[:])
            lt0 = proj_load(0)
            attn_chunk(1)
            lt1 = proj_load(1)
            attn_chunk(2)
            lt2 = proj_load(2)
            attn_chunk(3, head_hook=lambda h: proj_si(0, lt0, h - 1) if h > 0 else None)
            lt3 = proj_load(3)
            proj_si(0, lt0, 3)
            proj_chunk(1, lt1)
            proj_chunk(2, lt2)
            proj_chunk(3, lt3)
    lp.__exit__(None, None, None)
    nc.finalize()
    return nc


def _get_nc():
    if "nc" not in _CACHE:
        _CACHE["nc"] = _build_bass()
    return _CACHE["nc"]


def _make_in_maps(query, key, value, w_out):
    import ml_dtypes

    bf16 = ml_dtypes.bfloat16
    query = np.asarray(query, dtype=np.float32)
    key = np.asarray(key, dtype=np.float32)
    value = np.asarray(value, dtype=np.float32)
    w_out = np.asarray(w_out, dtype=np.float32)
    ones_b = np.ones((128, 128), dtype=bf16)
    ones_f = np.ones((1, 128), dtype=np.float32)
    in_maps = []
    for g in range(NCORES):
        # qT[p, (c h q)] = query[c*512+q, g*4+h, p]
        qTg = np.ascontiguousarray(
            query[:, g * HPG : (g + 1) * HPG, :]
            .reshape(NQC, QC, HPG, 128)
            .transpose(3, 0, 2, 1)
            .reshape(128, NQC * HPG * QC)
        ).astype(bf16)
        kTg = np.ascontiguousarray(key[:, g, :].T).astype(bf16)  # [D, S]
        # v[p, (t d)] = value[t*128+p, g, d]
        vg = np.ascontiguousarray(
            value[:, g, :].reshape(NKT, 128, D).transpose(1, 0, 2).reshape(128, NKT * D)
        ).astype(bf16)
        # wT[p, (a m)] = w_out[g*MS+m, a*128+p]
        wTg = np.ascontiguousarray(
            w_out[g * MS : (g + 1) * MS, :]
            .T.reshape(NJT, 128, MS)
            .transpose(1, 0, 2)
            .reshape(128, NJT * MS)
        ).astype(bf16)
        in_maps.append(
            {
                "qT": qTg,
                "kT": kTg,
                "v": vg,
                "wT": wTg,
                "ones_b": ones_b,
                "ones_f": ones_f,
            }
        )
    return in_maps


def run_sharded(query, key, value, w_out, trace=False):
    """Run the SPMD kernel; returns (out_full [S, MODEL], BassKernelResults)."""
    if "/opt/trn_rl_repo" not in sys.path:
        sys.path.insert(0, "/opt/trn_rl_repo")
    from concourse.bass_utils import run_bass_kernel_spmd

    nc = _get_nc()
    in_maps = _make_in_maps(query, key, value, w_out)
    res = run_bass_kernel_spmd(nc, in_maps, list(range(NCORES)), trace=trace)
    outs = [np.asarray(res.results[g]["out"]) for g in range(NCORES)]
    full = np.concatenate(outs, axis=1)  # [S, MODEL]
    return full, res


def kernel(query, key, value, mask, w_out, b_out):
    full, _ = run_sharded(query, key, value, w_out, trace=False)
    full = full + np.asarray(b_out, dtype=np.float32)[None, :]
    return full.reshape(S, H, D).astype(np.float32)

